# revision 1
# baseline (speedup 1.0000x reference)
"""DeepLSTM Trainium2 kernel: 2-layer LSTM (B=64,T=256,I=256,H=1024,O=256) on 8 cores.

Strategy: 8-way hidden-dim (gate) sharding. Core c owns hidden units
[c*128,(c+1)*128) of both LSTM layers and output cols [c*32,(c+1)*32).
Recurrent weights stay SBUF-resident. Hidden states are kept TRANSPOSED
(h^T: [hidden,batch]); each timestep the 8 h^T slices of each layer are
exchanged once per step with a single merged AllGather ([h1T(t+1) | h2T(t)],
bf16, 32KB/rank) — collectives have a large fixed cost here, so one per step.
The schedule is software-pipelined: when gather G(w) arrives, layer-1 of step
w+1 (the serial recurrence chain) issues first; layer-2 of step w and y of
step w-1 fill the gather window. Input projections (x @ W_ih^T) are
precomputed a few steps ahead inside the same loop (SBUF ring, float32r
matmuls). Recurrent matmuls run bf16 x bf16 with fp32 PSUM accumulation;
batch (64) rides as the stationary operand so the big weight operand streams
at 1 col/cycle. Measured on 8 axon-tunneled trn2 cores: ~7.4 ms/run
(wall-clock delta of in-kernel repetitions), max rel err vs fp32 jax
reference: 3.6e-4.

All host-side work is layout only (transposes / gather-index shuffles / bias
folding); every multiply-accumulate of the model runs on the NeuronCores.
"""
import numpy as np

import concourse.bacc as bacc
import concourse.mybir as mybir
import concourse.tile as tile
from concourse.bass_utils import run_bass_kernel_spmd
from concourse.masks import make_identity

B, T, I, H, O = 64, 256, 256, 1024, 256
P = 128           # partitions / hidden slice per core
NC = 8            # cores
GS = 512          # gate-slice cols per core (4 gates x 128)
OS = O // NC      # output cols per core (32)
LOOK = 10         # precompute lookahead (steps)
F32 = mybir.dt.float32
F32R = mybir.dt.float32r
BF16 = mybir.dt.bfloat16
ACT = mybir.ActivationFunctionType


def _r(ap):
    return ap.bitcast(F32R)


def build(nt=T, reps=1):
    tb = nt * B
    nc = bacc.Bacc("TRN2", num_devices=NC)
    XT = nc.dram_tensor("XT", [P, 2 * tb], F32R, kind="ExternalInput")
    W1s = nc.dram_tensor("W1s", [P, 8 * GS], BF16, kind="ExternalInput")
    W2a = nc.dram_tensor("W2a", [P, 8 * GS], BF16, kind="ExternalInput")
    W2b = nc.dram_tensor("W2b", [P, 8 * GS], BF16, kind="ExternalInput")
    Wy = nc.dram_tensor("Wy", [P, 8 * OS], BF16, kind="ExternalInput")
    W1x = nc.dram_tensor("W1x", [P, 2 * GS], F32R, kind="ExternalInput")
    W2x = nc.dram_tensor("W2x", [P, 2 * GS], F32R, kind="ExternalInput")
    Wyx = nc.dram_tensor("Wyx", [P, 2 * OS], F32R, kind="ExternalInput")
    bias1 = nc.dram_tensor("bias1", [B, GS], F32, kind="ExternalInput")
    bias2 = nc.dram_tensor("bias2", [B, GS], F32, kind="ExternalInput")
    biasy = nc.dram_tensor("biasy", [B, OS], F32, kind="ExternalInput")
    Yout = nc.dram_tensor("Yout", [B, nt * OS], F32, kind="ExternalOutput")
    cc_in = nc.dram_tensor("cc_in", [(nt + 1) * P, 2 * B], BF16)
    cc_out = nc.dram_tensor("cc_out", [(nt + 1) * NC * P, 2 * B], BF16,
                            addr_space="Shared")
    rg = [list(range(NC))]

    with tile.TileContext(nc) as tc:
        with (
            tc.tile_pool(name="wpool", bufs=1) as wpool,
            tc.tile_pool(name="state", bufs=1) as state,
            tc.tile_pool(name="xt", bufs=4) as xtp,
            tc.tile_pool(name="aring", bufs=LOOK + 4) as aring,
            tc.tile_pool(name="work", bufs=3) as work,
            tc.tile_pool(name="gath", bufs=3) as g1p,
            tc.tile_pool(name="gath2", bufs=3) as g2p,
            tc.tile_pool(name="psA", bufs=1, space="PSUM") as psA,
            tc.tile_pool(name="psG", bufs=1, space="PSUM") as psG,
            tc.tile_pool(name="psT", bufs=2, space="PSUM") as psT,
        ):
            # --- resident weights/biases ---
            w1s = wpool.tile([P, 8 * GS], BF16)
            w2a = wpool.tile([P, 8 * GS], BF16)
            w2b = wpool.tile([P, 8 * GS], BF16)
            wy = wpool.tile([P, 8 * OS], BF16)
            w1x = wpool.tile([P, 2 * GS], F32R)
            w2x = wpool.tile([P, 2 * GS], F32R)
            wyx = wpool.tile([P, 2 * OS], F32R)
            b1 = wpool.tile([B, GS], F32)
            b2 = wpool.tile([B, GS], F32)
            by = wpool.tile([B, OS], F32)
            ident = wpool.tile([B, B], F32)
            for k in range(8):
                nc.sync.dma_start(out=w1s[:, k * GS:(k + 1) * GS],
                                  in_=W1s[:, k * GS:(k + 1) * GS])
                nc.sync.dma_start(out=w2a[:, k * GS:(k + 1) * GS],
                                  in_=W2a[:, k * GS:(k + 1) * GS])
                nc.sync.dma_start(out=w2b[:, k * GS:(k + 1) * GS],
                                  in_=W2b[:, k * GS:(k + 1) * GS])
            nc.sync.dma_start(out=wy[:], in_=Wy[:])
            nc.sync.dma_start(out=w1x[:], in_=W1x[:])
            nc.sync.dma_start(out=w2x[:], in_=W2x[:])
            nc.sync.dma_start(out=wyx[:], in_=Wyx[:])
            nc.sync.dma_start(out=b1[:], in_=bias1[:])
            nc.sync.dma_start(out=b2[:], in_=bias2[:])
            nc.sync.dma_start(out=by[:], in_=biasy[:])
            make_identity(nc, ident[:])

            for _rep in range(reps):
                c1 = state.tile([B, P], F32, tag="c1")
                c2 = state.tile([B, P], F32, tag="c2")
                nc.gpsimd.memset(c1[:], 0.0)
                nc.gpsimd.memset(c2[:], 0.0)

                a1_t, a2_t, yx_t = {}, {}, {}

                def precompute(t):
                    xt = xtp.tile([P, 2 * B], F32R, tag="xt")
                    nc.scalar.dma_start(out=xt[:, 0:B], in_=XT[:, t * B:(t + 1) * B])
                    nc.scalar.dma_start(out=xt[:, B:2 * B],
                                        in_=XT[:, tb + t * B:tb + (t + 1) * B])
                    pa1 = psA.tile([B, GS], F32, tag="pa1")
                    pa2 = psA.tile([B, GS], F32, tag="pa2")
                    pyx = psA.tile([B, OS], F32, tag="pyx")
                    for k in range(2):
                        nc.tensor.matmul(pa1[:], lhsT=xt[:, k * B:(k + 1) * B],
                                         rhs=w1x[:, k * GS:(k + 1) * GS],
                                         start=(k == 0), stop=(k == 1))
                    for k in range(2):
                        nc.tensor.matmul(pa2[:], lhsT=xt[:, k * B:(k + 1) * B],
                                         rhs=w2x[:, k * GS:(k + 1) * GS],
                                         start=(k == 0), stop=(k == 1))
                    for k in range(2):
                        nc.tensor.matmul(pyx[:], lhsT=xt[:, k * B:(k + 1) * B],
                                         rhs=wyx[:, k * OS:(k + 1) * OS],
                                         start=(k == 0), stop=(k == 1))
                    a1 = aring.tile([B, GS], F32, tag="a1")
                    a2 = aring.tile([B, GS], F32, tag="a2")
                    yx = aring.tile([B, OS], F32, tag="yx")
                    nc.vector.tensor_add(a1[:], pa1[:], b1[:])
                    nc.vector.tensor_add(a2[:], pa2[:], b2[:])
                    nc.vector.tensor_add(yx[:], pyx[:], by[:])
                    a1_t[t], a2_t[t], yx_t[t] = a1, a2, yx

                def lstm_act(gsum, c, tag):
                    sig = work.tile([B, 384], F32, tag=f"sig{tag}")
                    tg = work.tile([B, P], F32, tag=f"tg{tag}")
                    nc.scalar.activation(sig[:], gsum[:, 0:384], ACT.Sigmoid)
                    nc.scalar.activation(tg[:], gsum[:, 384:512], ACT.Tanh)
                    t1 = work.tile([B, P], F32, tag=f"t1{tag}")
                    t2 = work.tile([B, P], F32, tag=f"t2{tag}")
                    nc.vector.tensor_mul(t1[:], sig[:, 0:P], tg[:])        # i*g
                    nc.vector.tensor_mul(t2[:], sig[:, P:2 * P], c[:])     # f*c
                    nc.vector.tensor_add(c[:], t1[:], t2[:])
                    tc_ = work.tile([B, P], F32, tag=f"tc{tag}")
                    nc.scalar.activation(tc_[:], c[:], ACT.Tanh)
                    h = work.tile([B, P], F32, tag=f"h{tag}")
                    nc.vector.tensor_mul(h[:], sig[:, 2 * P:3 * P], tc_[:])
                    return h

                def evict_T(h, stw, half):
                    """transpose h, cast bf16 into one half of the stage tile."""
                    tp = psT.tile([P, B], F32, tag="tps")
                    nc.tensor.transpose(tp[:], h[:], ident[:])
                    nc.vector.tensor_copy(stw[:, half * B:(half + 1) * B], tp[:])

                gath = {}

                def do_ag(w):
                    """AG window w: cc_in rows w -> gathered tile gath[w]."""
                    nc.gpsimd.collective_compute(
                        "AllGather", mybir.AluOpType.bypass,
                        ins=[cc_in[w * P:(w + 1) * P, :]],
                        outs=[cc_out[w * NC * P:(w + 1) * NC * P, :]],
                        replica_groups=rg)
                    g = g1p.tile([P, NC * 2 * B], BF16, tag="g")
                    cc_o = cc_out[w * NC * P:(w + 1) * NC * P, :].rearrange(
                        "(r p) f -> p r f", p=P)
                    gv = g[:].rearrange("p (r f) -> p r f", f=2 * B)
                    # h1 half on sync (gates critical layer-1 matmuls);
                    # h2 half in parallel on the scalar HWDGE engine
                    nc.sync.dma_start(out=gv[:, :, 0:B], in_=cc_o[:, :, 0:B])
                    nc.scalar.dma_start(out=gv[:, :, B:2 * B], in_=cc_o[:, :, B:2 * B])
                    gath[w] = g
                    return g

                def blk1(g, k):   # h1T block
                    return g[:, k * 2 * B: k * 2 * B + B]

                def blk2(g, k):   # h2T block
                    return g[:, k * 2 * B + B: (k + 1) * 2 * B]

                def layer1(t, gprev):
                    """h1(t) from h1T(t-1) in gprev; evict into cc window t."""
                    g1s = work.tile([B, GS], F32, tag="g1s")
                    if t == 0:
                        nc.vector.tensor_copy(g1s[:], a1_t.pop(t)[:])
                    else:
                        pg1 = psG.tile([B, GS], F32, tag="pg1")
                        for k in range(8):
                            nc.tensor.matmul(pg1[:], lhsT=blk1(gprev, k),
                                             rhs=w1s[:, k * GS:(k + 1) * GS],
                                             start=(k == 0), stop=(k == 7))
                        nc.vector.tensor_add(g1s[:], pg1[:], a1_t.pop(t)[:])
                    h1 = lstm_act(g1s, c1, "1")
                    evict_T(h1, stage_w[t], 0)

                for t in range(min(LOOK, nt)):
                    precompute(t)
                stage_w = {}

                def new_stage(w):
                    s = work.tile([P, 2 * B], BF16, tag="stw")
                    stage_w[w] = s
                    return s

                new_stage(0)
                layer1(0, None)
                nc.sync.dma_start(out=cc_in[0:P, 0:B], in_=stage_w[0][:, 0:B])
                do_ag(0)             # G(0) = [h1T(0) | garbage]

                for w in range(nt):
                    if w + LOOK < nt:
                        precompute(w + LOOK)
                    g = gath[w]
                    new_stage(w + 1)
                    # --- layer 1 of w+1 (critical chain) -> cc window w+1 ---
                    if w + 1 < nt:
                        layer1(w + 1, g)
                    # --- layer 2 of w ---
                    pg2 = psG.tile([B, GS], F32, tag="pg2")
                    nk = 16 if w > 0 else 8
                    ki = 0
                    for k in range(8):
                        nc.tensor.matmul(pg2[:], lhsT=blk1(g, k),
                                         rhs=w2a[:, k * GS:(k + 1) * GS],
                                         start=(ki == 0), stop=(ki == nk - 1))
                        ki += 1
                    if w > 0:
                        for k in range(8):
                            nc.tensor.matmul(pg2[:], lhsT=blk2(g, k),
                                             rhs=w2b[:, k * GS:(k + 1) * GS],
                                             start=(ki == 0), stop=(ki == nk - 1))
                            ki += 1
                    g2s = work.tile([B, GS], F32, tag="g2s")
                    nc.vector.tensor_add(g2s[:], pg2[:], a2_t.pop(w)[:])
                    h2 = lstm_act(g2s, c2, "2")
                    evict_T(h2, stage_w[w + 1], 1)   # h2T(w) -> stage
                    nc.sync.dma_start(out=cc_in[(w + 1) * P:(w + 2) * P, :],
                                      in_=stage_w[w + 1][:])
                    do_ag(w + 1)                 # G(w+1) = [h1T(w+1) | h2T(w)]
                    stage_w.pop(w, None)
                    # --- y(w-1) from h2T(w-1) in G(w) ---
                    if w > 0:
                        py = psG.tile([B, OS], F32, tag="py")
                        for k in range(8):
                            nc.tensor.matmul(py[:], lhsT=blk2(g, k),
                                             rhs=wy[:, k * OS:(k + 1) * OS],
                                             start=(k == 0), stop=(k == 7))
                        ys = work.tile([B, OS], F32, tag="ys")
                        nc.vector.tensor_add(ys[:], py[:], yx_t.pop(w - 1)[:])
                        nc.scalar.dma_start(out=Yout[:, (w - 1) * OS:w * OS], in_=ys[:])
                    if w > 0:
                        del gath[w - 1]

                # --- tail: y(nt-1) from h2T(nt-1) in G(nt) ---
                g = gath[nt]
                py = psG.tile([B, OS], F32, tag="py")
                for k in range(8):
                    nc.tensor.matmul(py[:], lhsT=blk2(g, k),
                                     rhs=wy[:, k * OS:(k + 1) * OS],
                                     start=(k == 0), stop=(k == 7))
                ys = work.tile([B, OS], F32, tag="ys")
                nc.vector.tensor_add(ys[:], py[:], yx_t.pop(nt - 1)[:])
                nc.sync.dma_start(out=Yout[:, (nt - 1) * OS:nt * OS], in_=ys[:])

    nc.finalize()
    return nc


def prep_inputs(inputs, nt=T):
    """Host-side layout prep -> per-core in_maps. Pure layout, no math beyond
    bias folding (b_ih + b_hh)."""
    x = np.ascontiguousarray(inputs["x"][:, :nt, :], np.float32)
    W_ih1 = np.asarray(inputs["W_ih1"], np.float32)
    W_hh1 = np.asarray(inputs["W_hh1"], np.float32)
    W_ih2 = np.asarray(inputs["W_ih2"], np.float32)
    W_hh2 = np.asarray(inputs["W_hh2"], np.float32)
    W_l = np.asarray(inputs["W_l"], np.float32)
    b1 = np.asarray(inputs["b_ih1"], np.float32) + np.asarray(inputs["b_hh1"], np.float32)
    b2 = np.asarray(inputs["b_ih2"], np.float32) + np.asarray(inputs["b_hh2"], np.float32)
    bl = np.asarray(inputs["b_l"], np.float32)

    tb = nt * B
    xt = np.ascontiguousarray(x.transpose(2, 1, 0))  # [I, T, B]
    XT = np.concatenate([xt[0:128].reshape(P, tb), xt[128:256].reshape(P, tb)],
                        axis=1)  # [128, 2*nt*B]

    in_maps = []
    for c in range(NC):
        hs = np.arange(c * P, (c + 1) * P)
        gate_idx = np.concatenate([hs + H * j for j in (0, 1, 3, 2)])  # i,f,o,g
        ys = np.arange(c * OS, (c + 1) * OS)

        def kblocks(Wt, n=8):  # Wt: [K, M] -> [128, n*M] k-block concat
            return np.concatenate([Wt[k * P:(k + 1) * P] for k in range(n)], axis=1)

        m = {
            "XT": XT,
            "W1s": kblocks(W_hh1[gate_idx].T.copy()),
            "W2a": kblocks(W_ih2[gate_idx, 256:].T.copy()),
            "W2b": kblocks(W_hh2[gate_idx].T.copy()),
            "Wy": kblocks(W_l[ys, 256:].T.copy()),
            "W1x": kblocks(W_ih1[gate_idx, :].T.copy(), 2),
            "W2x": kblocks(W_ih2[gate_idx, :256].T.copy(), 2),
            "Wyx": kblocks(W_l[ys, :256].T.copy(), 2),
            "bias1": np.tile(b1[gate_idx], (B, 1)),
            "bias2": np.tile(b2[gate_idx], (B, 1)),
            "biasy": np.tile(bl[ys], (B, 1)),
        }
        import ml_dtypes
        bf = {"W1s", "W2a", "W2b", "Wy"}
        in_maps.append({
            k: np.ascontiguousarray(v, ml_dtypes.bfloat16 if k in bf else np.float32)
            for k, v in m.items()})
    return in_maps


_cache = {}


def run(inputs, nt=T, reps=1):
    key = (nt, reps)
    if key not in _cache:
        _cache[key] = build(nt, reps)
    nc = _cache[key]
    in_maps = prep_inputs(inputs, nt)
    res = run_bass_kernel_spmd(nc, in_maps, core_ids=list(range(NC)))
    out = np.empty((B, nt, O), np.float32)
    for c in range(NC):
        out[:, :, c * OS:(c + 1) * OS] = res.results[c]["Yout"].reshape(B, nt, OS)
    return out


def kernel(**inputs) -> np.ndarray:
    return run(inputs, T)



# revision 2
# speedup vs baseline: 1.4635x; 1.4635x over previous
"""DeepLSTM Trainium2 kernel: 2-layer LSTM (B=64,T=256,I=256,H=1024,O=256) on 8 cores.

Strategy: 8-way hidden-dim (gate) sharding. Core c owns hidden units
[c*128,(c+1)*128) of both LSTM layers and output cols [c*32,(c+1)*32).
Recurrent weights stay SBUF-resident. Hidden states are kept TRANSPOSED
(h^T: [hidden,batch]); each timestep the 8 h^T slices of each layer are
exchanged once per step with a direct SBUF->SBUF remote-DMA broadcast
(remote_dma_broadcast to all 8 same-device peers, 32KB bf16 per step,
sender-indexed column slots via a partition-id register offset), replacing
the ncfw AllGather collective (~23us/step) with ~12us/step of SWDGE descgen
+ trigger + D2D flight + semaphore arrival waits. The schedule is
software-pipelined as before: when exchange w arrives, layer-1 of step w+1
(the serial recurrence chain) issues first; layer-2 of step w and y of step
w-1 fill the exchange window. Input projections (x @ W_ih^T) are precomputed
a few steps ahead inside the same loop (SBUF ring, float32r matmuls).
Recurrent matmuls run bf16 x bf16 with fp32 PSUM accumulation. Measured on
8 axon-tunneled trn2 cores: ~5.07 ms/run (wall-clock delta of in-kernel
repetitions), max rel err vs fp32 jax reference: 3.6e-4.

All host-side work is layout only (transposes / gather-index shuffles / bias
folding); every multiply-accumulate of the model runs on the NeuronCores.
"""
import numpy as np

import concourse.bacc as bacc
import concourse.mybir as mybir
import concourse.tile as tile
from concourse.bass import ds
from concourse.bass_utils import run_bass_kernel_spmd
from concourse.masks import make_identity

B, T, I, H, O = 64, 256, 256, 1024, 256
P = 128           # partitions / hidden slice per core
NC = 8            # cores
GS = 512          # gate-slice cols per core (4 gates x 128)
OS = O // NC      # output cols per core (32)
LOOK = 10         # precompute lookahead (steps)
RG = 4            # gather ring slots (remote-DMA allgather)
F32 = mybir.dt.float32
F32R = mybir.dt.float32r
BF16 = mybir.dt.bfloat16
ACT = mybir.ActivationFunctionType


def _r(ap):
    return ap.bitcast(F32R)


def build(nt=T, reps=1):
    tb = nt * B
    nc = bacc.Bacc("TRN2", num_devices=NC)
    XT = nc.dram_tensor("XT", [P, 2 * tb], F32R, kind="ExternalInput")
    W1s = nc.dram_tensor("W1s", [P, 8 * GS], BF16, kind="ExternalInput")
    W2a = nc.dram_tensor("W2a", [P, 8 * GS], BF16, kind="ExternalInput")
    W2b = nc.dram_tensor("W2b", [P, 8 * GS], BF16, kind="ExternalInput")
    Wy = nc.dram_tensor("Wy", [P, 8 * OS], BF16, kind="ExternalInput")
    W1x = nc.dram_tensor("W1x", [P, 2 * GS], F32R, kind="ExternalInput")
    W2x = nc.dram_tensor("W2x", [P, 2 * GS], F32R, kind="ExternalInput")
    Wyx = nc.dram_tensor("Wyx", [P, 2 * OS], F32R, kind="ExternalInput")
    bias1 = nc.dram_tensor("bias1", [B, GS], F32, kind="ExternalInput")
    bias2 = nc.dram_tensor("bias2", [B, GS], F32, kind="ExternalInput")
    biasy = nc.dram_tensor("biasy", [B, OS], F32, kind="ExternalInput")
    Yout = nc.dram_tensor("Yout", [B, nt * OS], F32, kind="ExternalOutput")

    arr_sem = nc.alloc_semaphore("rdma_arr")
    snd_sem = nc.alloc_semaphore("rdma_snd")
    prep_sem = nc.alloc_semaphore("rdma_prep")
    gp = nc.gpsimd

    with tile.TileContext(nc) as tc:
        with (
            tc.tile_pool(name="wpool", bufs=1) as wpool,
            tc.tile_pool(name="state", bufs=1) as state,
            tc.tile_pool(name="xt", bufs=4) as xtp,
            tc.tile_pool(name="aring", bufs=LOOK + 4) as aring,
            tc.tile_pool(name="work", bufs=3) as work,
            tc.tile_pool(name="psA", bufs=1, space="PSUM") as psA,
            tc.tile_pool(name="psG", bufs=1, space="PSUM") as psG,
            tc.tile_pool(name="psT", bufs=2, space="PSUM") as psT,
        ):
            # --- resident weights/biases ---
            w1s = wpool.tile([P, 8 * GS], BF16)
            w2a = wpool.tile([P, 8 * GS], BF16)
            w2b = wpool.tile([P, 8 * GS], BF16)
            wy = wpool.tile([P, 8 * OS], BF16)
            w1x = wpool.tile([P, 2 * GS], F32R)
            w2x = wpool.tile([P, 2 * GS], F32R)
            wyx = wpool.tile([P, 2 * OS], F32R)
            b1 = wpool.tile([B, GS], F32)
            b2 = wpool.tile([B, GS], F32)
            by = wpool.tile([B, OS], F32)
            ident = wpool.tile([B, B], F32)
            g_ring = [wpool.tile([P, NC * 2 * B], BF16, name=f"gr{r}")
                      for r in range(RG)]
            for k in range(8):
                nc.sync.dma_start(out=w1s[:, k * GS:(k + 1) * GS],
                                  in_=W1s[:, k * GS:(k + 1) * GS])
                nc.sync.dma_start(out=w2a[:, k * GS:(k + 1) * GS],
                                  in_=W2a[:, k * GS:(k + 1) * GS])
                nc.sync.dma_start(out=w2b[:, k * GS:(k + 1) * GS],
                                  in_=W2b[:, k * GS:(k + 1) * GS])
            nc.sync.dma_start(out=wy[:], in_=Wy[:])
            nc.sync.dma_start(out=w1x[:], in_=W1x[:])
            nc.sync.dma_start(out=w2x[:], in_=W2x[:])
            nc.sync.dma_start(out=wyx[:], in_=Wyx[:])
            nc.sync.dma_start(out=b1[:], in_=bias1[:])
            nc.sync.dma_start(out=b2[:], in_=bias2[:])
            nc.sync.dma_start(out=by[:], in_=biasy[:])
            make_identity(nc, ident[:])

            send_state = {"j": 0, "col": None}
            for _rep in range(reps):
                c1 = state.tile([B, P], F32, tag="c1")
                c2 = state.tile([B, P], F32, tag="c2")
                nc.gpsimd.memset(c1[:], 0.0)
                nc.gpsimd.memset(c2[:], 0.0)

                a1_t, a2_t, yx_t = {}, {}, {}

                def precompute(t):
                    xt = xtp.tile([P, 2 * B], F32R, tag="xt")
                    nc.scalar.dma_start(out=xt[:, 0:B], in_=XT[:, t * B:(t + 1) * B])
                    nc.scalar.dma_start(out=xt[:, B:2 * B],
                                        in_=XT[:, tb + t * B:tb + (t + 1) * B])
                    pa1 = psA.tile([B, GS], F32, tag="pa1")
                    pa2 = psA.tile([B, GS], F32, tag="pa2")
                    pyx = psA.tile([B, OS], F32, tag="pyx")
                    for k in range(2):
                        nc.tensor.matmul(pa1[:], lhsT=xt[:, k * B:(k + 1) * B],
                                         rhs=w1x[:, k * GS:(k + 1) * GS],
                                         start=(k == 0), stop=(k == 1))
                    for k in range(2):
                        nc.tensor.matmul(pa2[:], lhsT=xt[:, k * B:(k + 1) * B],
                                         rhs=w2x[:, k * GS:(k + 1) * GS],
                                         start=(k == 0), stop=(k == 1))
                    for k in range(2):
                        nc.tensor.matmul(pyx[:], lhsT=xt[:, k * B:(k + 1) * B],
                                         rhs=wyx[:, k * OS:(k + 1) * OS],
                                         start=(k == 0), stop=(k == 1))
                    a1 = aring.tile([B, GS], F32, tag="a1")
                    a2 = aring.tile([B, GS], F32, tag="a2")
                    yx = aring.tile([B, OS], F32, tag="yx")
                    nc.vector.tensor_add(a1[:], pa1[:], b1[:])
                    nc.vector.tensor_add(a2[:], pa2[:], b2[:])
                    nc.vector.tensor_add(yx[:], pyx[:], by[:])
                    a1_t[t], a2_t[t], yx_t[t] = a1, a2, yx

                def lstm_act(gsum, c, tag):
                    sig = work.tile([B, 384], F32, tag=f"sig{tag}")
                    tg = work.tile([B, P], F32, tag=f"tg{tag}")
                    nc.scalar.activation(sig[:], gsum[:, 0:384], ACT.Sigmoid)
                    nc.scalar.activation(tg[:], gsum[:, 384:512], ACT.Tanh)
                    t1 = work.tile([B, P], F32, tag=f"t1{tag}")
                    t2 = work.tile([B, P], F32, tag=f"t2{tag}")
                    nc.vector.tensor_mul(t1[:], sig[:, 0:P], tg[:])        # i*g
                    nc.vector.tensor_mul(t2[:], sig[:, P:2 * P], c[:])     # f*c
                    nc.vector.tensor_add(c[:], t1[:], t2[:])
                    tc_ = work.tile([B, P], F32, tag=f"tc{tag}")
                    nc.scalar.activation(tc_[:], c[:], ACT.Tanh)
                    h = work.tile([B, P], F32, tag=f"h{tag}")
                    nc.vector.tensor_mul(h[:], sig[:, 2 * P:3 * P], tc_[:])
                    return h

                def evict_T(h, stw, half):
                    """transpose h, cast bf16 into one half of the stage tile."""
                    tp = psT.tile([P, B], F32, tag="tps")
                    nc.tensor.transpose(tp[:], h[:], ident[:])
                    nc.vector.tensor_copy(stw[:, half * B:(half + 1) * B], tp[:])

                gath = {}

                def do_ag(w):
                    """Exchange window w: broadcast stage_w[w] ([h1T|h2T],
                    32KB bf16) SBUF->SBUF to all 8 cores' g_ring[w%RG] at
                    column slot pid*2B, then wait all 8 arrivals."""
                    j = send_state["j"]
                    g = g_ring[j % RG]
                    with tc.tile_critical(name=f"snd{j}"):
                        if j == 0:
                            gp.bir_kernel_barrier_wait([list(range(NC))])
                            send_state["col"] = gp.partition_id() * (2 * B)
                        if j >= 2:
                            # <=2 sends in flight; stage ring reuse safe
                            gp.wait_ge(snd_sem, 16 * (j - 1))
                        gp.remote_dma_broadcast(
                            out_ap=g[:, ds(send_state["col"], 2 * B)],
                            in_ap=stage_w[w][:],
                            remote_sem=arr_sem, local_sem=snd_sem,
                            rdests=[(0, k) for k in range(NC)],
                        ).then_inc(prep_sem, 1)
                        gp.wait_ge(prep_sem, j + 1)
                        gp.trigger_dma(count=1)
                        gp.wait_ge(arr_sem, 16 * (j + 1))
                    send_state["j"] = j + 1
                    gath[w] = g
                    return g

                def blk1(g, k):   # h1T block
                    return g[:, k * 2 * B: k * 2 * B + B]

                def blk2(g, k):   # h2T block
                    return g[:, k * 2 * B + B: (k + 1) * 2 * B]

                def layer1(t, gprev):
                    """h1(t) from h1T(t-1) in gprev; evict into cc window t."""
                    g1s = work.tile([B, GS], F32, tag="g1s")
                    if t == 0:
                        nc.vector.tensor_copy(g1s[:], a1_t.pop(t)[:])
                    else:
                        pg1 = psG.tile([B, GS], F32, tag="pg1")
                        for k in range(8):
                            nc.tensor.matmul(pg1[:], lhsT=blk1(gprev, k),
                                             rhs=w1s[:, k * GS:(k + 1) * GS],
                                             start=(k == 0), stop=(k == 7))
                        nc.vector.tensor_add(g1s[:], pg1[:], a1_t.pop(t)[:])
                    h1 = lstm_act(g1s, c1, "1")
                    evict_T(h1, stage_w[t], 0)

                for t in range(min(LOOK, nt)):
                    precompute(t)
                stage_w = {}

                def new_stage(w):
                    s = work.tile([P, 2 * B], BF16, tag="stw")
                    stage_w[w] = s
                    return s

                new_stage(0)
                layer1(0, None)
                nc.vector.memset(stage_w[0][:, B:2 * B], 0.0)
                do_ag(0)             # G(0) = [h1T(0) | zeros]

                for w in range(nt):
                    if w + LOOK < nt:
                        precompute(w + LOOK)
                    g = gath[w]
                    new_stage(w + 1)
                    # --- layer 1 of w+1 (critical chain) -> cc window w+1 ---
                    if w + 1 < nt:
                        layer1(w + 1, g)
                    # --- layer 2 of w ---
                    pg2 = psG.tile([B, GS], F32, tag="pg2")
                    nk = 16 if w > 0 else 8
                    ki = 0
                    for k in range(8):
                        nc.tensor.matmul(pg2[:], lhsT=blk1(g, k),
                                         rhs=w2a[:, k * GS:(k + 1) * GS],
                                         start=(ki == 0), stop=(ki == nk - 1))
                        ki += 1
                    if w > 0:
                        for k in range(8):
                            nc.tensor.matmul(pg2[:], lhsT=blk2(g, k),
                                             rhs=w2b[:, k * GS:(k + 1) * GS],
                                             start=(ki == 0), stop=(ki == nk - 1))
                            ki += 1
                    g2s = work.tile([B, GS], F32, tag="g2s")
                    nc.vector.tensor_add(g2s[:], pg2[:], a2_t.pop(w)[:])
                    h2 = lstm_act(g2s, c2, "2")
                    evict_T(h2, stage_w[w + 1], 1)   # h2T(w) -> stage
                    do_ag(w + 1)                 # G(w+1) = [h1T(w+1) | h2T(w)]
                    stage_w.pop(w, None)
                    # --- y(w-1) from h2T(w-1) in G(w) ---
                    if w > 0:
                        py = psG.tile([B, OS], F32, tag="py")
                        for k in range(8):
                            nc.tensor.matmul(py[:], lhsT=blk2(g, k),
                                             rhs=wy[:, k * OS:(k + 1) * OS],
                                             start=(k == 0), stop=(k == 7))
                        ys = work.tile([B, OS], F32, tag="ys")
                        nc.vector.tensor_add(ys[:], py[:], yx_t.pop(w - 1)[:])
                        nc.scalar.dma_start(out=Yout[:, (w - 1) * OS:w * OS], in_=ys[:])
                    gath.pop(w - 1, None)

                # --- tail: y(nt-1) from h2T(nt-1) in G(nt) ---
                g = gath[nt]
                py = psG.tile([B, OS], F32, tag="py")
                for k in range(8):
                    nc.tensor.matmul(py[:], lhsT=blk2(g, k),
                                     rhs=wy[:, k * OS:(k + 1) * OS],
                                     start=(k == 0), stop=(k == 7))
                ys = work.tile([B, OS], F32, tag="ys")
                nc.vector.tensor_add(ys[:], py[:], yx_t.pop(nt - 1)[:])
                nc.sync.dma_start(out=Yout[:, (nt - 1) * OS:nt * OS], in_=ys[:])

    nc.finalize()
    return nc


def prep_inputs(inputs, nt=T):
    """Host-side layout prep -> per-core in_maps. Pure layout, no math beyond
    bias folding (b_ih + b_hh)."""
    x = np.ascontiguousarray(inputs["x"][:, :nt, :], np.float32)
    W_ih1 = np.asarray(inputs["W_ih1"], np.float32)
    W_hh1 = np.asarray(inputs["W_hh1"], np.float32)
    W_ih2 = np.asarray(inputs["W_ih2"], np.float32)
    W_hh2 = np.asarray(inputs["W_hh2"], np.float32)
    W_l = np.asarray(inputs["W_l"], np.float32)
    b1 = np.asarray(inputs["b_ih1"], np.float32) + np.asarray(inputs["b_hh1"], np.float32)
    b2 = np.asarray(inputs["b_ih2"], np.float32) + np.asarray(inputs["b_hh2"], np.float32)
    bl = np.asarray(inputs["b_l"], np.float32)

    tb = nt * B
    xt = np.ascontiguousarray(x.transpose(2, 1, 0))  # [I, T, B]
    XT = np.concatenate([xt[0:128].reshape(P, tb), xt[128:256].reshape(P, tb)],
                        axis=1)  # [128, 2*nt*B]

    in_maps = []
    for c in range(NC):
        hs = np.arange(c * P, (c + 1) * P)
        gate_idx = np.concatenate([hs + H * j for j in (0, 1, 3, 2)])  # i,f,o,g
        ys = np.arange(c * OS, (c + 1) * OS)

        def kblocks(Wt, n=8):  # Wt: [K, M] -> [128, n*M] k-block concat
            return np.concatenate([Wt[k * P:(k + 1) * P] for k in range(n)], axis=1)

        m = {
            "XT": XT,
            "W1s": kblocks(W_hh1[gate_idx].T.copy()),
            "W2a": kblocks(W_ih2[gate_idx, 256:].T.copy()),
            "W2b": kblocks(W_hh2[gate_idx].T.copy()),
            "Wy": kblocks(W_l[ys, 256:].T.copy()),
            "W1x": kblocks(W_ih1[gate_idx, :].T.copy(), 2),
            "W2x": kblocks(W_ih2[gate_idx, :256].T.copy(), 2),
            "Wyx": kblocks(W_l[ys, :256].T.copy(), 2),
            "bias1": np.tile(b1[gate_idx], (B, 1)),
            "bias2": np.tile(b2[gate_idx], (B, 1)),
            "biasy": np.tile(bl[ys], (B, 1)),
        }
        import ml_dtypes
        bf = {"W1s", "W2a", "W2b", "Wy"}
        in_maps.append({
            k: np.ascontiguousarray(v, ml_dtypes.bfloat16 if k in bf else np.float32)
            for k, v in m.items()})
    return in_maps


_cache = {}


def run(inputs, nt=T, reps=1):
    key = (nt, reps)
    if key not in _cache:
        _cache[key] = build(nt, reps)
    nc = _cache[key]
    in_maps = prep_inputs(inputs, nt)
    res = run_bass_kernel_spmd(nc, in_maps, core_ids=list(range(NC)))
    out = np.empty((B, nt, O), np.float32)
    for c in range(NC):
        out[:, :, c * OS:(c + 1) * OS] = res.results[c]["Yout"].reshape(B, nt, OS)
    return out


def kernel(**inputs) -> np.ndarray:
    return run(inputs, T)

